# Initial kernel scaffold
#
"""Instant-NGP hash-encoding + tiny MLP on 8 TRN2 NeuronCores (Bass/Tile).

Strategy (data-parallel over points, tables replicated per core):
  - Levels 0..NDENSIFY-1: host pre-builds per-cell corner tables
    cc[cell] = 8 corners x 2 feats (64B). Device gathers ONE 64B row per
    point per level via [P,1] indirect DMA (one descriptor per partition).
  - Remaining (hash) levels: device computes the 8 spatial-hash corner
    indices and gathers 8B rows per corner via [P,1] indirect DMA.
  - Trilinear weights + weighted reduction on DVE; MLP (32->64 relu -> 8)
    on PE via per-512-point transposes; output written feature-major.
"""
import sys
sys.path.insert(0, "/opt/trn_rl_repo")
import numpy as np

LEVELS = 16
T = 2 ** 21
F = 2
N_POINTS = 524288
NCORES = 8
RES = [16, 22, 31, 42, 58, 81, 111, 154, 213, 294, 406, 562, 776, 1072, 1482, 2048]
DENSE_FORMULA = [l for l in range(LEVELS) if (RES[l] + 1) ** 3 <= T]   # 0..6
NDENSIFY = 9            # levels 0..8 use host-built cell tables
P = 128
P1 = 2654435761
P2 = 805459861
M = T - 1
P1_HI, P1_LO = P1 >> 16, P1 & 0xFFFF
P2_HI, P2_LO = P2 >> 16, P2 & 0xFFFF
AluOp = None  # set after mybir import


def _corner_bits(c):
    return (c >> 0) & 1, (c >> 1) & 1, (c >> 2) & 1


def build_cell_tables(tables):
    """Concat cell tables for levels 0..NDENSIFY-1.

    cc[l][cl, c*2+f] = tables[l][corner_index(cell, c), f],
    cl = x + res*y + res^2*z.
    Returns (concat [TOTCELLS, 16] f32, bases list in CELLS)."""
    parts, bases, base = [], [], 0
    for l in range(NDENSIFY):
        res = RES[l]
        ax = np.arange(res, dtype=np.int64)
        xg = ax[None, None, :]          # broadcast shapes [z, y, x]
        yg = ax[None, :, None]
        zg = ax[:, None, None]
        cc = np.empty((res, res, res, 8, F), dtype=np.float32)
        if l in DENSE_FORMULA:
            s = res + 1
            for c in range(8):
                dx, dy, dz = _corner_bits(c)
                idx = (xg + dx) + s * ((yg + dy) + s * (zg + dz))
                cc[..., c, :] = tables[l][idx]
        else:
            for c in range(8):
                dx, dy, dz = _corner_bits(c)
                h = ((xg + dx).astype(np.uint32)
                     ^ ((yg + dy).astype(np.uint32) * np.uint32(P1))
                     ^ ((zg + dz).astype(np.uint32) * np.uint32(P2))) & np.uint32(M)
                cc[..., c, :] = tables[l][h]
        # array is indexed [z, y, x] so flat order == x + res*y + res^2*z? No:
        # flat order of [z,y,x] is x fastest: flat = x + res*y + res^2*z. OK.
        parts.append(cc.reshape(res ** 3, 8 * F))
        bases.append(base)
        base += res ** 3
    return np.ascontiguousarray(np.concatenate(parts, axis=0)), bases


_cache = {}


def _build_program(npc):
    """Build + compile the per-core SPMD bass program for npc points/core."""
    import concourse.bass as bass
    import concourse.bacc as bacc
    import concourse.mybir as mybir
    import concourse.tile as tile
    from concourse.masks import make_identity

    Alu = mybir.AluOpType
    f32, i32 = mybir.dt.float32, mybir.dt.int32

    ccol = npc // P                      # free columns per core
    CH = min(256, ccol)                  # columns per chunk
    nchunk = ccol // CH
    assert nchunk * CH == ccol
    totcells = sum(RES[l] ** 3 for l in range(NDENSIFY))
    nhash = LEVELS - NDENSIFY
    hash_levels = list(range(NDENSIFY, LEVELS))

    nc = bacc.Bacc("TRN2", target_bir_lowering=False, debug=False,
                   num_devices=NCORES)

    pos_d = nc.dram_tensor("pos_t", [P, ccol, 3], f32, kind="ExternalInput")
    cc_d = nc.dram_tensor("cc", [totcells, 16], f32, kind="ExternalInput")
    ht_d = nc.dram_tensor("ht", [nhash * T, 2], f32, kind="ExternalInput")
    w0_d = nc.dram_tensor("w0", [32, 64], f32, kind="ExternalInput")
    b0_d = nc.dram_tensor("b0", [64, 1], f32, kind="ExternalInput")
    whd_d = nc.dram_tensor("whd", [64, 8], f32, kind="ExternalInput")
    bhd_d = nc.dram_tensor("bhd", [8, 1], f32, kind="ExternalInput")
    out_d = nc.dram_tensor("outT", [8, npc], f32, kind="ExternalOutput")

    cc_bases = []
    b = 0
    for l in range(NDENSIFY):
        cc_bases.append(b)
        b += RES[l] ** 3

    with tile.TileContext(nc) as tc:
        with tc.tile_pool(name="const", bufs=1) as cpool, \
             tc.tile_pool(name="work", bufs=2) as wpool, \
             tc.tile_pool(name="gath", bufs=2) as gpool, \
             tc.tile_pool(name="mlp", bufs=2) as mpool, \
             tc.tile_pool(name="psum", bufs=2, space="PSUM") as ppool:

            w0_sb = cpool.tile([32, 64], f32)
            nc.sync.dma_start(out=w0_sb[:], in_=w0_d.ap())
            b0_sb = cpool.tile([64, 1], f32)
            nc.sync.dma_start(out=b0_sb[:], in_=b0_d.ap())
            whd_sb = cpool.tile([64, 8], f32)
            nc.sync.dma_start(out=whd_sb[:], in_=whd_d.ap())
            bhd_sb = cpool.tile([8, 1], f32)
            nc.sync.dma_start(out=bhd_sb[:], in_=bhd_d.ap())
            ident = cpool.tile([P, P], f32)
            make_identity(nc, ident[:])

            for ch in range(nchunk):
                c0 = ch * CH
                pos = wpool.tile([P, CH, 3], f32, tag="pos")
                nc.sync.dma_start(out=pos[:], in_=pos_d.ap()[:, c0:c0 + CH, :])
                feats = wpool.tile([P, CH, 32], f32, tag="feats")

                for l in range(LEVELS):
                    res = RES[l]
                    # --- floor / frac per dim ---
                    xi, xf, wfr = [], [], []
                    for d in range(3):
                        ii = wpool.tile([P, CH], i32, tag=f"xi{d}")
                        # floor via round-to-nearest(t - 0.5); positions>=0
                        nc.vector.tensor_scalar(out=ii[:], in0=pos[:, :, d],
                                                scalar1=float(res), scalar2=-0.5,
                                                op0=Alu.mult, op1=Alu.add)
                        ff = wpool.tile([P, CH], f32, tag=f"xf{d}")
                        nc.vector.tensor_copy(out=ff[:], in_=ii[:])
                        ww = wpool.tile([P, CH], f32, tag=f"w{d}")
                        nc.vector.scalar_tensor_tensor(
                            out=ww[:], in0=pos[:, :, d], scalar=float(res),
                            in1=ff[:], op0=Alu.mult, op1=Alu.subtract)
                        xi.append(ii); xf.append(ff); wfr.append(ww)

                    # --- corner weights w8 [P, CH, 8] ---
                    ux = wpool.tile([P, CH], f32, tag="ux")
                    uy = wpool.tile([P, CH], f32, tag="uy")
                    uz = wpool.tile([P, CH], f32, tag="uz")
                    for u_, w_ in ((ux, wfr[0]), (uy, wfr[1]), (uz, wfr[2])):
                        nc.vector.tensor_scalar(out=u_[:], in0=w_[:],
                                                scalar1=-1.0, scalar2=1.0,
                                                op0=Alu.mult, op1=Alu.add)
                    wyz = []
                    for dz in (0, 1):
                        for dy in (0, 1):
                            t_ = wpool.tile([P, CH], f32, tag=f"wyz{dy}{dz}")
                            nc.vector.tensor_tensor(
                                out=t_[:], in0=(wfr[1] if dy else uy)[:],
                                in1=(wfr[2] if dz else uz)[:], op=Alu.mult)
                            wyz.append(t_)
                    w8 = wpool.tile([P, CH, 8], f32, tag="w8")
                    for c in range(8):
                        dx, dy, dz = _corner_bits(c)
                        nc.vector.tensor_tensor(
                            out=w8[:, :, c], in0=(wfr[0] if dx else ux)[:],
                            in1=wyz[dy + 2 * dz][:], op=Alu.mult)

                    gt = gpool.tile([P, CH, 16], f32, tag="g")
                    if l < NDENSIFY:
                        # cell index cl = x + res*(y + res*z) + base
                        a_ = wpool.tile([P, CH], i32, tag="cl_a")
                        nc.vector.tensor_scalar(out=a_[:], in0=xi[2][:],
                                                scalar1=res, scalar2=None,
                                                op0=Alu.mult)
                        b_ = wpool.tile([P, CH], i32, tag="cl_b")
                        nc.vector.tensor_tensor(out=b_[:], in0=a_[:],
                                                in1=xi[1][:], op=Alu.add)
                        c_ = wpool.tile([P, CH], i32, tag="cl_c")
                        nc.vector.tensor_scalar(out=c_[:], in0=b_[:],
                                                scalar1=res,
                                                scalar2=cc_bases[l],
                                                op0=Alu.mult, op1=Alu.add)
                        cl = wpool.tile([P, CH], i32, tag="cl")
                        nc.vector.tensor_tensor(out=cl[:], in0=c_[:],
                                                in1=xi[0][:], op=Alu.add)
                        for j in range(CH):
                            nc.gpsimd.indirect_dma_start(
                                out=gt[:, j, :], out_offset=None,
                                in_=cc_d.ap(),
                                in_offset=bass.IndirectOffsetOnAxis(
                                    ap=cl[:, j:j + 1], axis=0))
                    else:
                        # hash corners; yP/zP via 16-bit split mult (wrap-safe)
                        def mul_u32(src, hi, lo, tag):
                            m1 = wpool.tile([P, CH], i32, tag=tag + "m1")
                            nc.vector.tensor_scalar(out=m1[:], in0=src[:],
                                                    scalar1=hi, scalar2=0xFFFF,
                                                    op0=Alu.mult,
                                                    op1=Alu.bitwise_and)
                            m2 = wpool.tile([P, CH], i32, tag=tag + "m2")
                            nc.vector.tensor_scalar(out=m2[:], in0=m1[:],
                                                    scalar1=16, scalar2=None,
                                                    op0=Alu.logical_shift_left)
                            m3 = wpool.tile([P, CH], i32, tag=tag + "m3")
                            nc.vector.tensor_scalar(out=m3[:], in0=src[:],
                                                    scalar1=lo, scalar2=None,
                                                    op0=Alu.mult)
                            out_ = wpool.tile([P, CH], i32, tag=tag)
                            nc.vector.tensor_tensor(out=out_[:], in0=m2[:],
                                                    in1=m3[:], op=Alu.add)
                            return out_

                        yP = mul_u32(xi[1], P1_HI, P1_LO, "yP")
                        zP = mul_u32(xi[2], P2_HI, P2_LO, "zP")
                        yP1 = wpool.tile([P, CH], i32, tag="yP1")
                        nc.vector.tensor_scalar(out=yP1[:], in0=yP[:],
                                                scalar1=np.int32(P1 - (1 << 32)),
                                                scalar2=None, op0=Alu.add)
                        zP1 = wpool.tile([P, CH], i32, tag="zP1")
                        nc.vector.tensor_scalar(out=zP1[:], in0=zP[:],
                                                scalar1=np.int32(P2),
                                                scalar2=None, op0=Alu.add)
                        xi1 = wpool.tile([P, CH], i32, tag="xi1")
                        nc.vector.tensor_scalar(out=xi1[:], in0=xi[0][:],
                                                scalar1=1, scalar2=None,
                                                op0=Alu.add)
                        cyz = []
                        for dz in (0, 1):
                            for dy in (0, 1):
                                t_ = wpool.tile([P, CH], i32, tag=f"cyz{dy}{dz}")
                                nc.vector.tensor_tensor(
                                    out=t_[:], in0=(yP1 if dy else yP)[:],
                                    in1=(zP1 if dz else zP)[:],
                                    op=Alu.bitwise_xor)
                                cyz.append(t_)
                        e8 = wpool.tile([P, CH, 8], i32, tag="e8")
                        lbase = (l - NDENSIFY) * T
                        for c in range(8):
                            dx, dy, dz = _corner_bits(c)
                            h_ = wpool.tile([P, CH], i32, tag="h_")
                            nc.vector.tensor_tensor(
                                out=h_[:], in0=(xi1 if dx else xi[0])[:],
                                in1=cyz[dy + 2 * dz][:], op=Alu.bitwise_xor)
                            nc.vector.tensor_scalar(
                                out=e8[:, :, c], in0=h_[:], scalar1=M,
                                scalar2=lbase, op0=Alu.bitwise_and,
                                op1=Alu.add)
                        for j in range(CH):
                            for c in range(8):
                                nc.gpsimd.indirect_dma_start(
                                    out=gt[:, j, 2 * c:2 * c + 2],
                                    out_offset=None, in_=ht_d.ap(),
                                    in_offset=bass.IndirectOffsetOnAxis(
                                        ap=e8[:, j, c:c + 1], axis=0))

                    # --- weighted reduce into feats[:, :, 2l:2l+2] ---
                    prod = gpool.tile([P, CH, 8, 2], f32, tag="prod")
                    nc.vector.tensor_tensor(
                        out=prod[:],
                        in0=gt[:].rearrange("p c (k f) -> p c k f", f=2),
                        in1=w8[:].to_broadcast([P, CH, 8, 2]),
                        op=Alu.mult)
                    # reduce over corners: view [P, CH, 2, 8] then reduce X
                    pv = prod[:].rearrange("p c k f -> p c f k")
                    nc.vector.tensor_reduce(
                        out=feats[:, :, 2 * l:2 * l + 2], in_=pv,
                        axis=mybir.AxisListType.X, op=Alu.add)

                # ---- MLP over this chunk, 512 points per group ----
                ngrp = (P * CH) // 512      # groups of 4 columns
                for g in range(ngrp):
                    j0 = g * 4
                    px = ppool.tile([P, P], f32, tag="px")
                    nc.tensor.transpose(px[:], feats[:, j0:j0 + 4, :], ident[:])
                    sx = mpool.tile([P, P], f32, tag="sx")
                    nc.scalar.copy(out=sx[:], in_=px[:])
                    ph = ppool.tile([64, 512], f32, tag="ph")
                    for joff in range(4):
                        nc.tensor.matmul(
                            ph[:, joff * 128:(joff + 1) * 128],
                            w0_sb[:], sx[joff * 32:(joff + 1) * 32, :],
                            start=True, stop=True)
                    hsb = mpool.tile([64, 512], f32, tag="hsb")
                    nc.scalar.activation(
                        out=hsb[:], in_=ph[:],
                        func=mybir.ActivationFunctionType.Relu,
                        bias=b0_sb[:], scale=1.0)
                    po = ppool.tile([8, 512], f32, tag="po")
                    nc.tensor.matmul(po[:], whd_sb[:], hsb[:],
                                     start=True, stop=True)
                    osb = mpool.tile([8, 512], f32, tag="osb")
                    nc.vector.tensor_scalar(
                        out=osb[:], in0=po[:], scalar1=bhd_sb[:],
                        scalar2=None, op0=Alu.add)
                    # columns of this group: points (p, j0..j0+3) ->
                    # out flat id = p*ccol + (c0+j0+joff)
                    nc.sync.dma_start(
                        out=out_d.ap()[:, c0 + j0:c0 + j0 + 4]
                            .rearrange("o c -> o c 1"),
                        in_=osb[:].rearrange("o (j p) -> o j p", j=4),
                    )

    nc.compile()
    return nc


def _host_prep(positions, tables, w0, b0, wd, bd, wf, bf, npc):
    cc, _ = build_cell_tables(np.asarray(tables, dtype=np.float32))
    ht = np.ascontiguousarray(
        np.asarray(tables, dtype=np.float32)[NDENSIFY:].reshape(-1, 2))
    whd = np.concatenate([np.asarray(wf), np.asarray(wd)], axis=1).astype(np.float32)
    bhd = np.concatenate([np.asarray(bf), np.asarray(bd)]).astype(np.float32)[:, None]
    return cc, ht, whd, bhd


def kernel(positions, tables, w0, b0, wd, bd, wf, bf):
    positions = np.asarray(positions, dtype=np.float32)
    n = positions.shape[0]
    npc = n // NCORES
    ccol = npc // P

    from concourse.bass_utils import run_bass_kernel_spmd

    key = npc
    if key not in _cache:
        _cache[key] = _build_program(npc)
    nc = _cache[key]

    cc, ht, whd, bhd = _host_prep(positions, tables, w0, b0, wd, bd, wf, bf, npc)
    w0a = np.ascontiguousarray(np.asarray(w0, dtype=np.float32))
    b0a = np.asarray(b0, dtype=np.float32)[:, None]

    in_maps = []
    for k in range(NCORES):
        pk = positions[k * npc:(k + 1) * npc]          # [npc, 3]
        # device layout [P, ccol, 3]: point (p, j) = pk[p*ccol + j]
        pos_t = np.ascontiguousarray(pk.reshape(P, ccol, 3))
        in_maps.append({
            "pos_t": pos_t, "cc": cc, "ht": ht, "w0": w0a, "b0": b0a,
            "whd": whd, "bhd": bhd,
        })

    res = run_bass_kernel_spmd(nc, in_maps, core_ids=list(range(NCORES)))

    features = np.empty((n, 7), np.float32)
    density = np.empty((n, 1), np.float32)
    for k in range(NCORES):
        ot = res.results[k]["outT"]                    # [8, npc]
        ot = ot.reshape(8, npc)
        features[k * npc:(k + 1) * npc] = ot[:7].T
        density[k * npc:(k + 1) * npc] = ot[7:8].T
    return features, density


# revision 9
# speedup vs baseline: 1.0380x; 1.0380x over previous
"""Instant-NGP hash-encoding + tiny MLP on 8 TRN2 NeuronCores (Bass/Tile).

Strategy (data-parallel over points, tables replicated per core):
  - Levels 0..NDENSIFY-1: host pre-builds per-cell corner tables
    cc[cell] = 8 corners x 2 feats (64B). Device gathers ONE 64B row per
    point per level via [P,1] indirect DMA (one descriptor per partition).
  - Remaining (hash) levels: device computes the 8 spatial-hash corner
    indices and gathers 8B rows per corner via [P,1] indirect DMA.
  - Trilinear weights + weighted reduction on DVE; MLP (32->64 relu -> 8)
    on PE via per-512-point transposes; output written feature-major.
"""
import sys
sys.path.insert(0, "/opt/trn_rl_repo")
import numpy as np

LEVELS = 16
T = 2 ** 21
F = 2
N_POINTS = 524288
NCORES = 8
RES = [16, 22, 31, 42, 58, 81, 111, 154, 213, 294, 406, 562, 776, 1072, 1482, 2048]
DENSE_FORMULA = [l for l in range(LEVELS) if (RES[l] + 1) ** 3 <= T]   # 0..6
NDENSIFY = 9            # levels 0..8 use host-built cell tables
P = 128
P1 = 2654435761
P2 = 805459861
M = T - 1
P1_HI, P1_LO = P1 >> 16, P1 & 0xFFFF
P2_HI, P2_LO = P2 >> 16, P2 & 0xFFFF
AluOp = None  # set after mybir import
import os as _os
DEBUG_FEATS = _os.environ.get("DBGFEATS") == "1"


def _corner_bits(c):
    return (c >> 0) & 1, (c >> 1) & 1, (c >> 2) & 1


def build_cell_tables(tables):
    """Concat cell tables for levels 0..NDENSIFY-1.

    cc[l][cl, c*2+f] = tables[l][corner_index(cell, c), f],
    cl = x + res*y + res^2*z.
    Returns (concat [TOTCELLS, 16] f32, bases list in CELLS)."""
    parts, bases, base = [], [], 0
    for l in range(NDENSIFY):
        res = RES[l]
        ax = np.arange(res, dtype=np.int64)
        xg = ax[None, None, :]          # broadcast shapes [z, y, x]
        yg = ax[None, :, None]
        zg = ax[:, None, None]
        cc = np.empty((res, res, res, 8, F), dtype=np.float32)
        if l in DENSE_FORMULA:
            s = res + 1
            for c in range(8):
                dx, dy, dz = _corner_bits(c)
                idx = (xg + dx) + s * ((yg + dy) + s * (zg + dz))
                cc[..., c, :] = tables[l][idx]
        else:
            for c in range(8):
                dx, dy, dz = _corner_bits(c)
                h = ((xg + dx).astype(np.uint32)
                     ^ ((yg + dy).astype(np.uint32) * np.uint32(P1))
                     ^ ((zg + dz).astype(np.uint32) * np.uint32(P2))) & np.uint32(M)
                cc[..., c, :] = tables[l][h]
        # array is indexed [z, y, x] so flat order == x + res*y + res^2*z? No:
        # flat order of [z,y,x] is x fastest: flat = x + res*y + res^2*z. OK.
        parts.append(cc.reshape(res ** 3, 8 * F))
        bases.append(base)
        base += res ** 3
    return np.ascontiguousarray(np.concatenate(parts, axis=0)), bases


_cache = {}


def _build_program(npc):
    """Build + compile the per-core SPMD bass program for npc points/core."""
    import concourse.bass as bass
    import concourse.bacc as bacc
    import concourse.mybir as mybir
    import concourse.tile as tile
    from concourse.masks import make_identity

    Alu = mybir.AluOpType
    f32, i32 = mybir.dt.float32, mybir.dt.int32

    ccol = npc // P                      # free columns per core
    CH = min(256, ccol)                  # columns per chunk
    nchunk = ccol // CH
    assert nchunk * CH == ccol
    totcells = sum(RES[l] ** 3 for l in range(NDENSIFY))
    nhash = LEVELS - NDENSIFY
    hash_levels = list(range(NDENSIFY, LEVELS))

    nc = bacc.Bacc("TRN2", target_bir_lowering=False, debug=False,
                   num_devices=NCORES)

    pos_d = nc.dram_tensor("pos_t", [P, ccol, 3], f32, kind="ExternalInput")
    cc_d = nc.dram_tensor("cc", [totcells, 16], f32, kind="ExternalInput")
    ht_d = nc.dram_tensor("ht", [nhash * T, 2], f32, kind="ExternalInput")
    w0_d = nc.dram_tensor("w0", [32, 64], f32, kind="ExternalInput")
    b0_d = nc.dram_tensor("b0", [64, 1], f32, kind="ExternalInput")
    whd_d = nc.dram_tensor("whd", [64, 8], f32, kind="ExternalInput")
    bhd_d = nc.dram_tensor("bhd", [8, 1], f32, kind="ExternalInput")
    out_d = nc.dram_tensor("outT", [8, npc], f32, kind="ExternalOutput")
    fdbg_d = nc.dram_tensor("fdbg", [P, ccol, 32], f32, kind="ExternalOutput") if DEBUG_FEATS else None

    cc_bases = []
    b = 0
    for l in range(NDENSIFY):
        cc_bases.append(b)
        b += RES[l] ** 3

    with tile.TileContext(nc) as tc:
        with tc.tile_pool(name="const", bufs=1) as cpool, \
             tc.tile_pool(name="work", bufs=1) as wpool, \
             tc.tile_pool(name="feat", bufs=1) as fpool, \
             tc.tile_pool(name="gath", bufs=2) as gpool, \
             tc.tile_pool(name="mlp", bufs=2) as mpool, \
             tc.tile_pool(name="psum", bufs=2, space="PSUM") as ppool:

            w0_sb = cpool.tile([32, 64], f32)
            nc.sync.dma_start(out=w0_sb[:], in_=w0_d.ap())
            b0_sb = cpool.tile([64, 1], f32)
            nc.sync.dma_start(out=b0_sb[:], in_=b0_d.ap())
            whd_sb = cpool.tile([64, 8], f32)
            nc.sync.dma_start(out=whd_sb[:], in_=whd_d.ap())
            bhd_sb = cpool.tile([8, 1], f32)
            nc.sync.dma_start(out=bhd_sb[:], in_=bhd_d.ap())
            ident = cpool.tile([P, P], f32)
            make_identity(nc, ident[:])

            for ch in range(nchunk):
                c0 = ch * CH
                pos = wpool.tile([P, CH, 3], f32, tag="pos")
                nc.sync.dma_start(out=pos[:], in_=pos_d.ap()[:, c0:c0 + CH, :])
                feats = fpool.tile([P, CH, 32], f32, tag="feats")

                for l in range(LEVELS):
                    res = RES[l]
                    # --- floor / frac per dim ---
                    xi, xf, wfr = [], [], []
                    for d in range(3):
                        ii = wpool.tile([P, CH], i32, tag=f"xi{d}")
                        # floor via round-to-nearest(t - 0.5); positions>=0
                        nc.vector.tensor_scalar(out=ii[:], in0=pos[:, :, d],
                                                scalar1=float(res), scalar2=-0.5,
                                                op0=Alu.mult, op1=Alu.add)
                        ff = wpool.tile([P, CH], f32, tag=f"xf{d}")
                        nc.vector.tensor_copy(out=ff[:], in_=ii[:])
                        ww = wpool.tile([P, CH], f32, tag=f"w{d}")
                        nc.vector.scalar_tensor_tensor(
                            out=ww[:], in0=pos[:, :, d], scalar=float(res),
                            in1=ff[:], op0=Alu.mult, op1=Alu.subtract)
                        xi.append(ii); xf.append(ff); wfr.append(ww)

                    # --- corner weights w8 [P, CH, 8] ---
                    ux = wpool.tile([P, CH], f32, tag="ux")
                    uy = wpool.tile([P, CH], f32, tag="uy")
                    uz = wpool.tile([P, CH], f32, tag="uz")
                    for u_, w_ in ((ux, wfr[0]), (uy, wfr[1]), (uz, wfr[2])):
                        nc.vector.tensor_scalar(out=u_[:], in0=w_[:],
                                                scalar1=-1.0, scalar2=1.0,
                                                op0=Alu.mult, op1=Alu.add)
                    wyz = []
                    for dz in (0, 1):
                        for dy in (0, 1):
                            t_ = wpool.tile([P, CH], f32, tag=f"wyz{dy}{dz}")
                            nc.vector.tensor_tensor(
                                out=t_[:], in0=(wfr[1] if dy else uy)[:],
                                in1=(wfr[2] if dz else uz)[:], op=Alu.mult)
                            wyz.append(t_)
                    w8 = wpool.tile([P, CH, 8], f32, tag="w8")
                    for c in range(8):
                        dx, dy, dz = _corner_bits(c)
                        nc.vector.tensor_tensor(
                            out=w8[:, :, c], in0=(wfr[0] if dx else ux)[:],
                            in1=wyz[dy + 2 * dz][:], op=Alu.mult)

                    gt = gpool.tile([P, CH, 16], f32, tag="g")
                    if l < NDENSIFY:
                        # cell index cl = x + res*(y + res*z) + base
                        a_ = wpool.tile([P, CH], i32, tag="cl_a")
                        nc.vector.tensor_scalar(out=a_[:], in0=xi[2][:],
                                                scalar1=res, scalar2=None,
                                                op0=Alu.mult)
                        b_ = wpool.tile([P, CH], i32, tag="cl_b")
                        nc.vector.tensor_tensor(out=b_[:], in0=a_[:],
                                                in1=xi[1][:], op=Alu.add)
                        c_ = wpool.tile([P, CH], i32, tag="cl_c")
                        nc.vector.tensor_scalar(out=c_[:], in0=b_[:],
                                                scalar1=res,
                                                scalar2=cc_bases[l],
                                                op0=Alu.mult, op1=Alu.add)
                        cl = wpool.tile([P, CH], i32, tag="cl")
                        nc.vector.tensor_tensor(out=cl[:], in0=c_[:],
                                                in1=xi[0][:], op=Alu.add)
                        for j in range(CH):
                            nc.gpsimd.indirect_dma_start(
                                out=gt[:, j, :], out_offset=None,
                                in_=cc_d.ap(),
                                in_offset=bass.IndirectOffsetOnAxis(
                                    ap=cl[:, j:j + 1], axis=0))
                    else:
                        # hash corners; yP/zP via 16-bit split mult (wrap-safe)
                        def mul_u32(src, hi, lo, tag):
                            m0 = wpool.tile([P, CH], i32, tag=tag + "m0")
                            nc.vector.tensor_scalar(out=m0[:], in0=src[:],
                                                    scalar1=hi, scalar2=None,
                                                    op0=Alu.mult)
                            m1 = wpool.tile([P, CH], i32, tag=tag + "m1")
                            nc.vector.tensor_scalar(out=m1[:], in0=m0[:],
                                                    scalar1=0xFFFF, scalar2=None,
                                                    op0=Alu.bitwise_and)
                            m2 = wpool.tile([P, CH], i32, tag=tag + "m2")
                            nc.vector.tensor_scalar(out=m2[:], in0=m1[:],
                                                    scalar1=16, scalar2=None,
                                                    op0=Alu.logical_shift_left)
                            m3 = wpool.tile([P, CH], i32, tag=tag + "m3")
                            nc.vector.tensor_scalar(out=m3[:], in0=src[:],
                                                    scalar1=lo, scalar2=None,
                                                    op0=Alu.mult)
                            out_ = wpool.tile([P, CH], i32, tag=tag)
                            nc.vector.tensor_tensor(out=out_[:], in0=m2[:],
                                                    in1=m3[:], op=Alu.add)
                            return out_

                        yP = mul_u32(xi[1], P1_HI, P1_LO, "yP")
                        zP = mul_u32(xi[2], P2_HI, P2_LO, "zP")
                        yP1 = wpool.tile([P, CH], i32, tag="yP1")
                        nc.vector.tensor_scalar(out=yP1[:], in0=yP[:],
                                                scalar1=P1 - (1 << 32),
                                                scalar2=None, op0=Alu.add)
                        zP1 = wpool.tile([P, CH], i32, tag="zP1")
                        nc.vector.tensor_scalar(out=zP1[:], in0=zP[:],
                                                scalar1=P2 if P2 < (1 << 31) else P2 - (1 << 32),
                                                scalar2=None, op0=Alu.add)
                        xi1 = wpool.tile([P, CH], i32, tag="xi1")
                        nc.vector.tensor_scalar(out=xi1[:], in0=xi[0][:],
                                                scalar1=1, scalar2=None,
                                                op0=Alu.add)
                        cyz = []
                        for dz in (0, 1):
                            for dy in (0, 1):
                                t_ = wpool.tile([P, CH], i32, tag=f"cyz{dy}{dz}")
                                nc.vector.tensor_tensor(
                                    out=t_[:], in0=(yP1 if dy else yP)[:],
                                    in1=(zP1 if dz else zP)[:],
                                    op=Alu.bitwise_xor)
                                cyz.append(t_)
                        e8 = wpool.tile([P, CH, 8], i32, tag="e8")
                        lbase = (l - NDENSIFY) * T
                        for c in range(8):
                            dx, dy, dz = _corner_bits(c)
                            h_ = wpool.tile([P, CH], i32, tag="h_")
                            nc.vector.tensor_tensor(
                                out=h_[:], in0=(xi1 if dx else xi[0])[:],
                                in1=cyz[dy + 2 * dz][:], op=Alu.bitwise_xor)
                            hm = wpool.tile([P, CH], i32, tag="hm")
                            nc.vector.tensor_scalar(
                                out=hm[:], in0=h_[:], scalar1=M,
                                scalar2=None, op0=Alu.bitwise_and)
                            nc.vector.tensor_scalar(
                                out=e8[:, :, c], in0=hm[:], scalar1=lbase,
                                scalar2=None, op0=Alu.add)
                        for j in range(CH):
                            for c in range(8):
                                nc.gpsimd.indirect_dma_start(
                                    out=gt[:, j, 2 * c:2 * c + 2],
                                    out_offset=None, in_=ht_d.ap(),
                                    in_offset=bass.IndirectOffsetOnAxis(
                                        ap=e8[:, j, c:c + 1], axis=0))

                    # --- weighted reduce into feats[:, :, 2l:2l+2] ---
                    gv = gt[:].rearrange("p c (k f) -> p c k f", f=2)
                    nc.vector.tensor_tensor(
                        out=gv, in0=gv,
                        in1=w8[:].to_broadcast([P, CH, 8, 2]),
                        op=Alu.mult)
                    # reduce over corners: view [P, CH, 2, 8] then reduce X
                    pv = gt[:].rearrange("p c (k f) -> p c f k", f=2)
                    nc.vector.tensor_reduce(
                        out=feats[:, :, 2 * l:2 * l + 2], in_=pv,
                        axis=mybir.AxisListType.X, op=Alu.add)

                if DEBUG_FEATS:
                    nc.sync.dma_start(out=fdbg_d.ap()[:, c0:c0 + CH, :],
                                      in_=feats[:])
                # ---- MLP over this chunk, 512 points per group ----
                ngrp = (P * CH) // 512      # groups of 4 columns
                for g in range(ngrp):
                    j0 = g * 4
                    px = ppool.tile([32, 512], f32, tag="px")
                    for joff in range(4):
                        nc.tensor.transpose(
                            px[:, joff * 128:(joff + 1) * 128],
                            feats[:, j0 + joff, :], ident[:])
                    sx = mpool.tile([32, 512], f32, tag="sx")
                    nc.scalar.copy(out=sx[:], in_=px[:])
                    ph = ppool.tile([64, 512], f32, tag="ph")
                    nc.tensor.matmul(ph[:], w0_sb[:], sx[:],
                                     start=True, stop=True)
                    hsb = mpool.tile([64, 512], f32, tag="hsb")
                    nc.scalar.activation(
                        out=hsb[:], in_=ph[:],
                        func=mybir.ActivationFunctionType.Relu,
                        bias=b0_sb[:], scale=1.0)
                    po = ppool.tile([8, 512], f32, tag="po")
                    nc.tensor.matmul(po[:], whd_sb[:], hsb[:],
                                     start=True, stop=True)
                    osb = mpool.tile([8, 512], f32, tag="osb")
                    nc.vector.tensor_scalar(
                        out=osb[:], in0=po[:], scalar1=bhd_sb[:],
                        scalar2=None, op0=Alu.add)
                    # device-natural order: out col = ch*CH*128 + g*512
                    #   + joff*128 + p  (host unscrambles)
                    off = ch * CH * P + g * 512
                    nc.sync.dma_start(
                        out=out_d.ap()[:, off:off + 512], in_=osb[:])

    nc.compile()
    return nc


def _host_prep(positions, tables, w0, b0, wd, bd, wf, bf, npc):
    cc, _ = build_cell_tables(np.asarray(tables, dtype=np.float32))
    ht = np.ascontiguousarray(
        np.asarray(tables, dtype=np.float32)[NDENSIFY:].reshape(-1, 2))
    whd = np.concatenate([np.asarray(wf), np.asarray(wd)], axis=1).astype(np.float32)
    bhd = np.concatenate([np.asarray(bf), np.asarray(bd)]).astype(np.float32)[:, None]
    return cc, ht, whd, bhd


def kernel(positions, tables, w0, b0, wd, bd, wf, bf):
    positions = np.asarray(positions, dtype=np.float32)
    n = positions.shape[0]
    npc = n // NCORES
    ccol = npc // P

    from concourse.bass_utils import run_bass_kernel_spmd

    key = npc
    if key not in _cache:
        _cache[key] = _build_program(npc)
    nc = _cache[key]

    cc, ht, whd, bhd = _host_prep(positions, tables, w0, b0, wd, bd, wf, bf, npc)
    w0a = np.ascontiguousarray(np.asarray(w0, dtype=np.float32))
    b0a = np.asarray(b0, dtype=np.float32)[:, None]

    in_maps = []
    for k in range(NCORES):
        pk = positions[k * npc:(k + 1) * npc]          # [npc, 3]
        # device layout [P, ccol, 3]: point (p, j) = pk[p*ccol + j]
        pos_t = np.ascontiguousarray(pk.reshape(P, ccol, 3))
        in_maps.append({
            "pos_t": pos_t, "cc": cc, "ht": ht, "w0": w0a, "b0": b0a,
            "whd": whd, "bhd": bhd,
        })

    res = run_bass_kernel_spmd(nc, in_maps, core_ids=list(range(NCORES)))

    # device out col s = ch*CH*128 + g*512 + joff*128 + p maps to point
    # id = p*ccol + ch*CH + g*4 + joff
    CH = min(256, ccol)
    nchunk = ccol // CH
    ngrp = (P * CH) // 512
    s_ch, s_g, s_joff, s_p = np.meshgrid(
        np.arange(nchunk), np.arange(ngrp), np.arange(4), np.arange(P),
        indexing="ij")
    point_id = (s_p * ccol + s_ch * CH + s_g * 4 + s_joff).reshape(-1)

    if DEBUG_FEATS:
        np.save("/tmp/act_f32.npy",
                np.stack([res.results[k]["fdbg"] for k in range(NCORES)]))
    features = np.empty((n, 7), np.float32)
    density = np.empty((n, 1), np.float32)
    for k in range(NCORES):
        ot = res.results[k]["outT"].reshape(8, npc)    # [8, device-col]
        fb = features[k * npc:(k + 1) * npc]
        db = density[k * npc:(k + 1) * npc]
        fb[point_id] = ot[:7].T
        db[point_id] = ot[7:8].T
    return features, density
